# revision 7
# baseline (speedup 1.0000x reference)
"""Trainium2 Bass kernel for CustomTemporalAttention.

B=8, T=1024, E=1024, H=16, D=64. Sharding: pure batch data-parallel across the
8 NeuronCores (core b computes batch element b end-to-end; weights and the tiny
bias table are replicated). No collectives.

Per-core math (torch Linear convention x @ W.T + b):
  qT = Wq @ query[b].T  (stored transposed: [E, T], channel-major)
  kT likewise; v in [T, E] layout augmented with a ones column per head.
  Per head h: S^T[tk, tq] = kT_h.T-contract = sum_d kT[d,tk] qT[d,tq]
  P^T = exp(0.125 * (S^T + 8*biasT)) via DVE add + ACT exp(scale=0.125)
  [num; den] = [v_h | 1].T @ P^T  accumulated over tk chunks (PSUM [65, 512])
  O^T_h = num / den ; y = O @ Wo.T + bo.

Temporal bias: bias(q,k) = lerp(table[q - k + T-1]) with a global fractional
shift u = tanh(offset)/2.  blend[r] = a*tabp[r] + b*tabp[r+1] + c*tabp[r+2]
with a=relu(-u), b=1-|u|, c=relu(u) and tabp edge-padded — exact for the
clipped endpoints. We need Toeplitz tiles blend[C' - i + j]; materialized per
head as BSp[i, y] = rblend[y + i] (one overlapping-window DMA from DRAM) and
read back with reversed free-dim slices (both patterns hardware-verified).
"""

import sys

sys.path.insert(0, "/opt/trn_rl_repo")

import numpy as np

import concourse.bass as bass
import concourse.mybir as mybir
import concourse.tile as tile
from concourse.bass_utils import run_bass_kernel_spmd

F32 = mybir.dt.float32
F32R = mybir.dt.float32r
AF = mybir.ActivationFunctionType
ALU = mybir.AluOpType

B, T, E, H = 8, 1024, 1024, 16
D = E // H  # 64
TQ = 512  # query-tile width (free dim of S^T matmuls)
W_BSP = 1920  # per-head shifted-blend slab width


def _split_multi_waits(nc):
    """This walrus build accepts at most one sync-wait per instruction; hoist
    extras onto same-engine NoOp carriers placed immediately before."""
    n = 0
    for f in nc.m.functions:
        for blk in f.blocks:
            out = []
            for inst in blk.instructions:
                si = inst.sync_info
                waits = list(si.on_wait) if si and si.on_wait else []
                if len(waits) > 1:
                    for w in waits[:-1]:
                        n += 1
                        nop = mybir.InstNoOp(name=f"{inst.name}-ws{n}", ins=[], outs=[])
                        nop.engine = inst.engine
                        nop.sync_info = mybir.SyncInfo(on_wait=[w], on_update=[])
                        out.append(nop)
                    inst.sync_info = mybir.SyncInfo(
                        on_wait=[waits[-1]], on_update=list(si.on_update or [])
                    )
                out.append(inst)
            blk.instructions = out
    return n


def _craft(ap, dims, offset=None):
    c = ap.copy()
    c.ap = ap.ap.__class__(dims)
    if offset is not None:
        c.offset = offset
    return c


def _build():
    nc = bass.Bass()

    # --- I/O (per-core shard; same program on all 8 cores) ---
    xqT = nc.declare_dram_parameter("xqT", [E, T], F32R, isOutput=False)
    xkT = nc.declare_dram_parameter("xkT", [E, T], F32R, isOutput=False)
    xvT = nc.declare_dram_parameter("xvT", [E, T], F32R, isOutput=False)
    wqT = nc.declare_dram_parameter("wqT", [E, E], F32R, isOutput=False)
    wkT = nc.declare_dram_parameter("wkT", [E, E], F32R, isOutput=False)
    wvT = nc.declare_dram_parameter("wvT", [E, E], F32R, isOutput=False)
    woT = nc.declare_dram_parameter("woT", [E, E], F32R, isOutput=False)
    bq2 = nc.declare_dram_parameter("bq2", [128, 8], F32, isOutput=False)
    bk2 = nc.declare_dram_parameter("bk2", [128, 8], F32, isOutput=False)
    bv1 = nc.declare_dram_parameter("bv1", [E], F32, isOutput=False)
    bo1 = nc.declare_dram_parameter("bo1", [E], F32, isOutput=False)
    rtabp = nc.declare_dram_parameter("rtabp", [H, 2 * T + 1], F32, isOutput=False)
    offs = nc.declare_dram_parameter("offs", [1], F32, isOutput=False)
    y_out = nc.declare_dram_parameter("y", [T, E], F32, isOutput=True)

    with tile.TileContext(nc) as tc:
        with (
            tc.tile_pool(name="persist", bufs=1) as persist,
            tc.tile_pool(name="small", bufs=1) as small,
            tc.tile_pool(name="dram", bufs=1, space="DRAM") as drp,
        ):
            # ---- persistent SBUF tensors ----
            qT = persist.tile([128, 8, T], F32R, tag="qT")
            kT = persist.tile([128, 8, T], F32R, tag="kT")
            vp = persist.tile([128, 8, H, D + 1], F32R, tag="vp")
            oT = persist.tile([128, 8, T], F32R, tag="oT")
            bvrep = persist.tile([128, E], F32, tag="bvrep")
            borep = persist.tile([128, E], F32, tag="borep")
            bqs = small.tile([128, 8], F32, tag="bqs")
            bks = small.tile([128, 8], F32, tag="bks")

            nc.sync.dma_start(out=bqs[:], in_=bq2[:])
            nc.sync.dma_start(out=bks[:], in_=bk2[:])
            # broadcast bv/bo across partitions (zero-step DRAM src)
            nc.sync.dma_start(out=bvrep[:], in_=_craft(bv1[:], [[0, 128], [1, E]], 0))
            nc.sync.dma_start(out=borep[:], in_=_craft(bo1[:], [[0, 128], [1, E]], 0))

            # ---- phase 0: blended relative-position table ----
            p0ctx = tc.tile_pool(name="p0", bufs=1)
            p0 = p0ctx.__enter__()
            tab = p0.tile([H, 2 * T + 1], F32, tag="tab")
            nc.sync.dma_start(out=tab[:], in_=rtabp[:])
            off_sb = p0.tile([1, 1], F32, tag="off")
            nc.sync.dma_start(out=off_sb[:], in_=offs[None, :])
            th = p0.tile([1, 1], F32, tag="th")
            nc.scalar.activation(th[:], off_sb[:], AF.Tanh)
            w8 = p0.tile([1, 1], F32, tag="w8")
            nc.vector.tensor_scalar_mul(w8[:], th[:], 4.0)  # 8*u = 4*tanh
            abc = p0.tile([1, 3], F32, tag="abc")
            # a = relu(-8u), c = relu(8u), b = 8 - a - c
            nc.vector.tensor_scalar(abc[:, 0:1], w8[:], -1.0, 0.0, ALU.mult, ALU.max)
            nc.vector.tensor_scalar(abc[:, 2:3], w8[:], 1.0, 0.0, ALU.mult, ALU.max)
            tsum = p0.tile([1, 1], F32, tag="tsum")
            nc.vector.tensor_tensor(tsum[:], abc[:, 0:1], abc[:, 2:3], ALU.add)
            nc.vector.tensor_scalar(abc[:, 1:2], tsum[:], -1.0, 8.0, ALU.mult, ALU.add)
            abc_dram = drp.tile([3], F32, tag="abc_dram")
            nc.sync.dma_start(out=abc_dram[None, :], in_=abc[:])
            abc16 = p0.tile([H, 3], F32, tag="abc16")
            nc.sync.dma_start(out=abc16[:], in_=_craft(abc_dram[:], [[0, H], [1, 3]], 0))

            # rblend8[h, r] = a*rtabp[r+2] + b*rtabp[r+1] + c*rtabp[r]   (x8 folded in)
            nblend = 2 * T - 1
            rb = p0.tile([H, nblend], F32, tag="rb")
            rb_t = p0.tile([H, nblend], F32, tag="rb_t")
            nc.vector.tensor_scalar(rb[:], tab[:, 2 : 2 + nblend], abc16[:, 0:1], None, ALU.mult)
            nc.vector.tensor_scalar(rb_t[:], tab[:, 1 : 1 + nblend], abc16[:, 1:2], None, ALU.mult)
            nc.vector.tensor_tensor(rb[:], rb[:], rb_t[:], ALU.add)
            nc.vector.tensor_scalar(rb_t[:], tab[:, 0:nblend], abc16[:, 2:3], None, ALU.mult)
            nc.vector.tensor_tensor(rb[:], rb[:], rb_t[:], ALU.add)
            rb_dram = drp.tile([H, nblend], F32, tag="rb_dram")
            nc.sync.dma_start(out=rb_dram[:], in_=rb[:])
            p0ctx.__exit__(None, None, None)

            # ---- phase 1: projections ----
            with (
                tc.tile_pool(name="xt", bufs=1) as xtp,
                tc.tile_pool(name="wt", bufs=3) as wtp,
                tc.tile_pool(name="wtv", bufs=1) as wtvp,
                tc.tile_pool(name="pps", bufs=4, space="PSUM") as pps,
            ):
                # q and k -> transposed layout [f-chunk partitions, t free]
                for name, x_in, w_in, dst, bias_sb in (
                    ("q", xqT, wqT, qT, bqs),
                    ("k", xkT, wkT, kT, bks),
                ):
                    xt = []
                    for eo in range(8):
                        for tq in range(2):
                            t_ = xtp.tile([128, TQ], F32R, tag=f"xt{eo}_{tq}")
                            nc.sync.dma_start(
                                out=t_[:],
                                in_=x_in[128 * eo : 128 * eo + 128, TQ * tq : TQ * tq + TQ],
                            )
                            xt.append(t_)
                    for fo in range(8):
                        # two psum accumulation groups in parallel so each
                        # streamed weight tile is consumed immediately
                        ps = [pps.tile([128, TQ], F32, tag="pps", name=f"pp{fo}_{i}") for i in range(2)]
                        for eo in range(8):
                            wt_ = wtp.tile([128, 128], F32R, tag="wt")
                            nc.sync.dma_start(
                                out=wt_[:],
                                in_=w_in[128 * eo : 128 * eo + 128, 128 * fo : 128 * fo + 128],
                            )
                            for tq in range(2):
                                nc.tensor.matmul(
                                    ps[tq][:],
                                    wt_[:],
                                    xt[2 * eo + tq][:],
                                    start=(eo == 0),
                                    stop=(eo == 7),
                                )
                        for tq in range(2):
                            nc.vector.tensor_scalar(
                                dst[:, fo, TQ * tq : TQ * tq + TQ],
                                ps[tq][:],
                                1.0,
                                bias_sb[:, fo : fo + 1],
                                ALU.mult,
                                ALU.add,
                            )

                # v -> [t-chunk partitions, head-blocked free] + ones col
                xt = []
                for eo in range(8):
                    for to2 in range(2):
                        t_ = xtp.tile([128, TQ], F32R, tag=f"xt{eo}_{to2}")
                        nc.sync.dma_start(
                            out=t_[:],
                            in_=xvT[128 * eo : 128 * eo + 128, TQ * to2 : TQ * to2 + TQ],
                        )
                        xt.append(t_)
                for fv in range(2):
                    wts = []
                    for eo in range(8):
                        wt_ = wtvp.tile([128, TQ], F32R, tag=f"wtv{eo}")
                        nc.sync.dma_start(
                            out=wt_[:],
                            in_=wvT[128 * eo : 128 * eo + 128, TQ * fv : TQ * fv + TQ],
                        )
                        wts.append(wt_)
                    for to in range(8):
                        to2, toi = divmod(to, 4)
                        ps = pps.tile([128, TQ], F32, tag="pps")
                        for eo in range(8):
                            nc.tensor.matmul(
                                ps[:],
                                xt[2 * eo + to2][:, 128 * toi : 128 * toi + 128],
                                wts[eo][:],
                                start=(eo == 0),
                                stop=(eo == 7),
                            )
                        nc.vector.tensor_tensor(
                            vp[:, to, 8 * fv : 8 * fv + 8, 0:D],
                            ps[:].rearrange("p (h d) -> p h d", d=D),
                            bvrep[:, TQ * fv : TQ * fv + TQ].rearrange(
                                "p (h d) -> p h d", d=D
                            ),
                            ALU.add,
                        )
                # ones columns
                nc.vector.tensor_scalar(
                    vp[:, :, :, D : D + 1], vp[:, :, :, D : D + 1], 0.0, 1.0, ALU.mult, ALU.add
                )

            # ---- phase 2: attention ----
            with (
                tc.tile_pool(name="bsp", bufs=2) as bspp,
                tc.tile_pool(name="pt", bufs=10) as ptp,
                tc.tile_pool(name="sm", bufs=4) as smp,
                tc.tile_pool(name="sps", bufs=4, space="PSUM") as sps,
                tc.tile_pool(name="ops", bufs=2, space="PSUM") as ops,
                tc.tile_pool(name="dr2", bufs=2, space="DRAM") as drp2,
            ):
                for h in range(H):
                    hp0 = 64 * (h % 2)
                    po = h // 2
                    bsp = bspp.tile([128, W_BSP], F32, tag="bsp")
                    nc.sync.dma_start(
                        out=bsp[:],
                        in_=_craft(rb_dram[:], [[1, 128], [1, W_BSP]], h * nblend),
                    )
                    for tq in range(2):
                        opsum = ops.tile([D + 1, TQ], F32, tag="ops")
                        for c in range(8):
                            spsum = sps.tile([128, TQ], F32, tag="sps")
                            nc.tensor.matmul(
                                spsum[:],
                                kT[hp0 : hp0 + 64, po, 128 * c : 128 * c + 128],
                                qT[hp0 : hp0 + 64, po, TQ * tq : TQ * tq + TQ],
                                start=True,
                                stop=True,
                            )
                            s = 1023 + 128 * c - TQ * tq
                            nc.vector.tensor_tensor(
                                spsum[:],
                                spsum[:],
                                bsp[:, s - (TQ - 1) : s + 1][:, ::-1],
                                ALU.add,
                            )
                            pt = ptp.tile([128, TQ], F32R, tag="pt")
                            nc.scalar.activation(pt[:], spsum[:], AF.Exp, scale=0.125)
                            nc.tensor.matmul(
                                opsum[:],
                                vp[:, c, h, :],
                                pt[:],
                                start=(c == 0),
                                stop=(c == 7),
                            )
                        rec = smp.tile([1, TQ], F32, tag="rec")
                        nc.vector.reciprocal(rec[:], opsum[D : D + 1, :])
                        rec_dram = drp2.tile([TQ], F32, tag="recd")
                        nc.sync.dma_start(out=rec_dram[None, :], in_=rec[:])
                        rep = smp.tile([64, TQ], F32, tag="rep")
                        nc.sync.dma_start(
                            out=rep[:], in_=_craft(rec_dram[:], [[0, 64], [1, TQ]], 0)
                        )
                        if hp0 == 0:
                            nc.vector.tensor_tensor(
                                oT[0:64, po, TQ * tq : TQ * tq + TQ],
                                opsum[0:D, :],
                                rep[:],
                                ALU.mult,
                            )
                        else:
                            onrm = smp.tile([64, TQ], F32R, tag="onrm")
                            nc.vector.tensor_tensor(
                                onrm[:], opsum[0:D, :], rep[:], ALU.mult
                            )
                            nc.sync.dma_start(
                                out=oT[64:128, po, TQ * tq : TQ * tq + TQ], in_=onrm[:]
                            )

            # ---- phase 3: output projection ----
            with (
                tc.tile_pool(name="wo", bufs=1) as wop,
                tc.tile_pool(name="yst", bufs=4) as ystp,
                tc.tile_pool(name="pps3", bufs=4, space="PSUM") as pps3,
            ):
                for fo2 in range(2):
                    wts = []
                    for co in range(8):
                        wt_ = wop.tile([128, TQ], F32R, tag=f"wo{co}")
                        nc.sync.dma_start(
                            out=wt_[:],
                            in_=woT[128 * co : 128 * co + 128, TQ * fo2 : TQ * fo2 + TQ],
                        )
                        wts.append(wt_)
                    for to in range(8):
                        ps = pps3.tile([128, TQ], F32, tag="pps3")
                        for co in range(8):
                            nc.tensor.matmul(
                                ps[:],
                                oT[:, co, 128 * to : 128 * to + 128],
                                wts[co][:],
                                start=(co == 0),
                                stop=(co == 7),
                            )
                        yst = ystp.tile([128, TQ], F32, tag="yst")
                        nc.vector.tensor_tensor(
                            yst[:], ps[:], borep[:, TQ * fo2 : TQ * fo2 + TQ], ALU.add
                        )
                        nc.sync.dma_start(
                            out=y_out[128 * to : 128 * to + 128, TQ * fo2 : TQ * fo2 + TQ],
                            in_=yst[:],
                        )

    _split_multi_waits(nc)
    return nc


_NC_CACHE = None


def _get_nc():
    global _NC_CACHE
    if _NC_CACHE is None:
        _NC_CACHE = _build()
    return _NC_CACHE


def _prepare_in_maps(
    query, key_, value, Wq, bq, Wk, bk, Wv, bv, Wo, bo, bias_table, offset
):
    query = np.asarray(query, np.float32)
    key_ = np.asarray(key_, np.float32)
    value = np.asarray(value, np.float32)
    shared = {
        "wqT": np.ascontiguousarray(np.asarray(Wq, np.float32).T),
        "wkT": np.ascontiguousarray(np.asarray(Wk, np.float32).T),
        "wvT": np.ascontiguousarray(np.asarray(Wv, np.float32).T),
        "woT": np.ascontiguousarray(np.asarray(Wo, np.float32).T),
        "bq2": np.ascontiguousarray(np.asarray(bq, np.float32).reshape(8, 128).T),
        "bk2": np.ascontiguousarray(np.asarray(bk, np.float32).reshape(8, 128).T),
        "bv1": np.ascontiguousarray(np.asarray(bv, np.float32)),
        "bo1": np.ascontiguousarray(np.asarray(bo, np.float32)),
        "offs": np.ascontiguousarray(np.asarray(offset, np.float32)),
    }
    tab = np.asarray(bias_table, np.float32)  # [2T-1, H]
    pad = np.concatenate([tab[0:1], tab, tab[-1:]], axis=0)  # [2T+1, H]
    shared["rtabp"] = np.ascontiguousarray(pad[::-1].T)  # [H, 2T+1]

    in_maps = []
    for b in range(B):
        m = dict(shared)
        m["xqT"] = np.ascontiguousarray(query[b].T)
        m["xkT"] = np.ascontiguousarray(key_[b].T)
        m["xvT"] = np.ascontiguousarray(value[b].T)
        in_maps.append(m)
    return in_maps


def kernel(**inputs):
    inputs.pop("key", None)
    in_maps = _prepare_in_maps(
        inputs["query"], inputs["key_"], inputs["value"],
        inputs["Wq"], inputs["bq"], inputs["Wk"], inputs["bk"],
        inputs["Wv"], inputs["bv"], inputs["Wo"], inputs["bo"],
        inputs["bias_table"], inputs["offset"],
    )
    nc = _get_nc()
    res = run_bass_kernel_spmd(nc, in_maps, list(range(B)))
    out = np.stack([res.results[b]["y"] for b in range(B)], axis=0)
    return out.astype(np.float32)


# revision 9
# speedup vs baseline: 1.2677x; 1.2677x over previous
"""Trainium2 Bass kernel for CustomTemporalAttention.

B=8, T=1024, E=1024, H=16, D=64. Sharding: pure batch data-parallel across the
8 NeuronCores (core b computes batch element b end-to-end; weights and the tiny
bias table are replicated). No collectives.

Per-core math (torch Linear convention x @ W.T + b):
  qT = Wq @ query[b].T  (stored transposed: [E, T], channel-major)
  kT likewise; v in [T, E] layout augmented with a ones column per head.
  Per head h: S^T[tk, tq] = sum_d kT[d,tk] qT[d,tq]
  P^T = exp(0.125 * (S^T + 8*biasT)) via DVE add + ACT exp(scale=0.125)
  [num; den] = [v_h | 1].T @ P^T  accumulated over tk chunks (PSUM [65, 512])
  O^T_h = num / den ; y = O @ Wo.T + bo.

Matmul operands are bf16 (fp32 PSUM accumulate): full-chain numpy model gives
5.1e-3 max rel err vs the fp32 reference. bf16 restores fast-weight-load and
LDWEIGHTS/ MATMUL overlap that fp32/f32r modes forfeit.

Temporal bias: bias(q,k) = lerp(table[q - k + T-1]) with a global fractional
shift u = tanh(offset)/2.  blend[r] = a*tabp[r] + b*tabp[r+1] + c*tabp[r+2]
with a=relu(-u), b=1-|u|, c=relu(u) and tabp edge-padded — exact including the
clipped endpoints. Toeplitz tiles blend[C' - i + j] are materialized per head
as BSp[i, y] = rblend[y + i] (one overlapping-window DMA from DRAM scratch)
and read back with reversed free-dim slices (both patterns HW-verified).
"""

import sys

sys.path.insert(0, "/opt/trn_rl_repo")

import ml_dtypes
import numpy as np

import concourse.bass as bass
import concourse.mybir as mybir
import concourse.tile as tile
from concourse.bass_utils import run_bass_kernel_spmd

F32 = mybir.dt.float32
BF16 = mybir.dt.bfloat16
AF = mybir.ActivationFunctionType
ALU = mybir.AluOpType

B, T, E, H = 8, 1024, 1024, 16
D = E // H  # 64
TQ = 512
W_BSP = 1920


def _split_multi_waits(nc):
    """This walrus build accepts at most one sync-wait per instruction; hoist
    extras onto same-engine NoOp carriers placed immediately before."""
    n = 0
    for f in nc.m.functions:
        for blk in f.blocks:
            out = []
            for inst in blk.instructions:
                si = inst.sync_info
                waits = list(si.on_wait) if si and si.on_wait else []
                if len(waits) > 1:
                    for w in waits[:-1]:
                        n += 1
                        nop = mybir.InstNoOp(name=f"{inst.name}-ws{n}", ins=[], outs=[])
                        nop.engine = inst.engine
                        nop.sync_info = mybir.SyncInfo(on_wait=[w], on_update=[])
                        out.append(nop)
                    inst.sync_info = mybir.SyncInfo(
                        on_wait=[waits[-1]], on_update=list(si.on_update or [])
                    )
                out.append(inst)
            blk.instructions = out
    return n


def _craft(ap, dims, offset=None):
    c = ap.copy()
    c.ap = ap.ap.__class__(dims)
    if offset is not None:
        c.offset = offset
    return c


def _build():
    nc = bass.Bass()

    xqT = nc.declare_dram_parameter("xqT", [E, T], BF16, isOutput=False)
    xkT = nc.declare_dram_parameter("xkT", [E, T], BF16, isOutput=False)
    xvT = nc.declare_dram_parameter("xvT", [E, T], BF16, isOutput=False)
    wqT = nc.declare_dram_parameter("wqT", [E, E], BF16, isOutput=False)
    wkT = nc.declare_dram_parameter("wkT", [E, E], BF16, isOutput=False)
    wvT = nc.declare_dram_parameter("wvT", [E, E], BF16, isOutput=False)
    woT = nc.declare_dram_parameter("woT", [E, E], BF16, isOutput=False)
    bq2 = nc.declare_dram_parameter("bq2", [128, 8], F32, isOutput=False)
    bk2 = nc.declare_dram_parameter("bk2", [128, 8], F32, isOutput=False)
    bv1 = nc.declare_dram_parameter("bv1", [E], F32, isOutput=False)
    bo1 = nc.declare_dram_parameter("bo1", [E], F32, isOutput=False)
    rtabp = nc.declare_dram_parameter("rtabp", [H, 2 * T + 1], F32, isOutput=False)
    offs = nc.declare_dram_parameter("offs", [1], F32, isOutput=False)
    y_out = nc.declare_dram_parameter("y", [T, E], F32, isOutput=True)

    with tile.TileContext(nc) as tc:
        with (
            tc.tile_pool(name="persist", bufs=1) as persist,
            tc.tile_pool(name="small", bufs=1) as small,
            tc.tile_pool(name="dram", bufs=1, space="DRAM") as drp,
        ):
            qT = persist.tile([128, 8, T], BF16, tag="qT")
            kT = persist.tile([128, 8, T], BF16, tag="kT")
            vp = persist.tile([128, 8, H, D + 1], BF16, tag="vp")
            oT = persist.tile([128, 8, T], BF16, tag="oT")
            bvrep = persist.tile([128, E], F32, tag="bvrep")
            borep = persist.tile([128, E], F32, tag="borep")
            bqs = small.tile([128, 8], F32, tag="bqs")
            bks = small.tile([128, 8], F32, tag="bks")

            nc.sync.dma_start(out=bqs[:], in_=bq2[:])
            nc.sync.dma_start(out=bks[:], in_=bk2[:])
            nc.sync.dma_start(out=bvrep[:], in_=_craft(bv1[:], [[0, 128], [1, E]], 0))
            nc.sync.dma_start(out=borep[:], in_=_craft(bo1[:], [[0, 128], [1, E]], 0))

            # ---- phase 0: blended relative-position table ----
            p0ctx = tc.tile_pool(name="p0", bufs=1)
            p0 = p0ctx.__enter__()
            tab = p0.tile([H, 2 * T + 1], F32, tag="tab")
            nc.sync.dma_start(out=tab[:], in_=rtabp[:])
            off_sb = p0.tile([1, 1], F32, tag="off")
            nc.sync.dma_start(out=off_sb[:], in_=offs[None, :])
            th = p0.tile([1, 1], F32, tag="th")
            nc.scalar.activation(th[:], off_sb[:], AF.Tanh)
            w8 = p0.tile([1, 1], F32, tag="w8")
            nc.vector.tensor_scalar_mul(w8[:], th[:], 4.0)  # 8*u = 4*tanh
            abc = p0.tile([1, 3], F32, tag="abc")
            nc.vector.tensor_scalar(abc[:, 0:1], w8[:], -1.0, 0.0, ALU.mult, ALU.max)
            nc.vector.tensor_scalar(abc[:, 2:3], w8[:], 1.0, 0.0, ALU.mult, ALU.max)
            tsum = p0.tile([1, 1], F32, tag="tsum")
            nc.vector.tensor_tensor(tsum[:], abc[:, 0:1], abc[:, 2:3], ALU.add)
            nc.vector.tensor_scalar(abc[:, 1:2], tsum[:], -1.0, 8.0, ALU.mult, ALU.add)
            abc_dram = drp.tile([3], F32, tag="abc_dram")
            nc.sync.dma_start(out=abc_dram[None, :], in_=abc[:])
            abc16 = p0.tile([H, 3], F32, tag="abc16")
            nc.sync.dma_start(out=abc16[:], in_=_craft(abc_dram[:], [[0, H], [1, 3]], 0))

            nblend = 2 * T - 1
            rb = p0.tile([H, nblend], F32, tag="rb")
            rb_t = p0.tile([H, nblend], F32, tag="rb_t")
            nc.vector.tensor_scalar(rb[:], tab[:, 2 : 2 + nblend], abc16[:, 0:1], None, ALU.mult)
            nc.vector.tensor_scalar(rb_t[:], tab[:, 1 : 1 + nblend], abc16[:, 1:2], None, ALU.mult)
            nc.vector.tensor_tensor(rb[:], rb[:], rb_t[:], ALU.add)
            nc.vector.tensor_scalar(rb_t[:], tab[:, 0:nblend], abc16[:, 2:3], None, ALU.mult)
            nc.vector.tensor_tensor(rb[:], rb[:], rb_t[:], ALU.add)
            rb_dram = drp.tile([H, nblend], F32, tag="rb_dram")
            nc.sync.dma_start(out=rb_dram[:], in_=rb[:])
            p0ctx.__exit__(None, None, None)

            # ---- phase 1: projections ----
            with (
                tc.tile_pool(name="xt", bufs=2) as xtp,
                tc.tile_pool(name="wt", bufs=10) as wtp,
                tc.tile_pool(name="wtv", bufs=1) as wtvp,
                tc.tile_pool(name="pps", bufs=4, space="PSUM") as pps,
            ):
                for name, x_in, w_in, dst, bias_sb in (
                    ("q", xqT, wqT, qT, bqs),
                    ("k", xkT, wkT, kT, bks),
                ):
                    xt = []
                    for eo in range(8):
                        for tq in range(2):
                            t_ = xtp.tile([128, TQ], BF16, tag=f"xt{eo}_{tq}")
                            nc.sync.dma_start(
                                out=t_[:],
                                in_=x_in[128 * eo : 128 * eo + 128, TQ * tq : TQ * tq + TQ],
                            )
                            xt.append(t_)
                    for fo in range(8):
                        ps = [pps.tile([128, TQ], F32, tag="pps", name=f"pp{fo}_{i}") for i in range(2)]
                        for eo in range(8):
                            wt_ = wtp.tile([128, 128], BF16, tag="wt")
                            nc.sync.dma_start(
                                out=wt_[:],
                                in_=w_in[128 * eo : 128 * eo + 128, 128 * fo : 128 * fo + 128],
                            )
                            for tq in range(2):
                                nc.tensor.matmul(
                                    ps[tq][:],
                                    wt_[:],
                                    xt[2 * eo + tq][:],
                                    start=(eo == 0),
                                    stop=(eo == 7),
                                )
                        for tq in range(2):
                            nc.vector.tensor_scalar(
                                dst[:, fo, TQ * tq : TQ * tq + TQ],
                                ps[tq][:],
                                1.0,
                                bias_sb[:, fo : fo + 1],
                                ALU.mult,
                                ALU.add,
                            )

                xt = []
                for eo in range(8):
                    for to2 in range(2):
                        t_ = xtp.tile([128, TQ], BF16, tag=f"xt{eo}_{to2}")
                        nc.sync.dma_start(
                            out=t_[:],
                            in_=xvT[128 * eo : 128 * eo + 128, TQ * to2 : TQ * to2 + TQ],
                        )
                        xt.append(t_)
                for fv in range(2):
                    wts = []
                    for eo in range(8):
                        wt_ = wtvp.tile([128, TQ], BF16, tag=f"wtv{eo}")
                        nc.sync.dma_start(
                            out=wt_[:],
                            in_=wvT[128 * eo : 128 * eo + 128, TQ * fv : TQ * fv + TQ],
                        )
                        wts.append(wt_)
                    for to in range(8):
                        to2, toi = divmod(to, 4)
                        ps = pps.tile([128, TQ], F32, tag="pps")
                        for eo in range(8):
                            nc.tensor.matmul(
                                ps[:],
                                xt[2 * eo + to2][:, 128 * toi : 128 * toi + 128],
                                wts[eo][:],
                                start=(eo == 0),
                                stop=(eo == 7),
                            )
                        nc.vector.tensor_tensor(
                            vp[:, to, 8 * fv : 8 * fv + 8, 0:D],
                            ps[:].rearrange("p (h d) -> p h d", d=D),
                            bvrep[:, TQ * fv : TQ * fv + TQ].rearrange(
                                "p (h d) -> p h d", d=D
                            ),
                            ALU.add,
                        )
                nc.vector.memset(vp[:, :, :, D : D + 1], 1.0)

            # ---- phase 2: attention ----
            with (
                tc.tile_pool(name="bsp", bufs=2) as bspp,
                tc.tile_pool(name="pt", bufs=10) as ptp,
                tc.tile_pool(name="sm", bufs=4) as smp,
                tc.tile_pool(name="sps", bufs=4, space="PSUM") as sps,
                tc.tile_pool(name="ops", bufs=4, space="PSUM") as ops,
                tc.tile_pool(name="dr2", bufs=2, space="DRAM") as drp2,
            ):
                for h in range(H):
                    hp0 = 64 * (h % 2)
                    po = h // 2
                    bsp = bspp.tile([128, W_BSP], F32, tag="bsp")
                    nc.sync.dma_start(
                        out=bsp[:],
                        in_=_craft(rb_dram[:], [[1, 128], [1, W_BSP]], h * nblend),
                    )
                    opsum = [
                        ops.tile([D + 1, TQ], F32, tag="ops", name=f"op{h}_{i}")
                        for i in range(2)
                    ]
                    for c in range(8):
                        for tq in range(2):
                            spsum = sps.tile([128, TQ], F32, tag="sps")
                            nc.tensor.matmul(
                                spsum[:],
                                kT[hp0 : hp0 + 64, po, 128 * c : 128 * c + 128],
                                qT[hp0 : hp0 + 64, po, TQ * tq : TQ * tq + TQ],
                                start=True,
                                stop=True,
                            )
                            s = 1023 + 128 * c - TQ * tq
                            nc.vector.tensor_tensor(
                                spsum[:],
                                spsum[:],
                                bsp[:, s - (TQ - 1) : s + 1][:, ::-1],
                                ALU.add,
                            )
                            pt = ptp.tile([128, TQ], BF16, tag="pt")
                            nc.scalar.activation(pt[:], spsum[:], AF.Exp, scale=0.125)
                            nc.tensor.matmul(
                                opsum[tq][:],
                                vp[:, c, h, :],
                                pt[:],
                                start=(c == 0),
                                stop=(c == 7),
                            )
                    for tq in range(2):
                        rec = smp.tile([1, TQ], F32, tag="rec")
                        nc.vector.reciprocal(rec[:], opsum[tq][D : D + 1, :])
                        rec_dram = drp2.tile([TQ], F32, tag="recd")
                        nc.sync.dma_start(out=rec_dram[None, :], in_=rec[:])
                        rep = smp.tile([64, TQ], F32, tag="rep")
                        nc.sync.dma_start(
                            out=rep[:], in_=_craft(rec_dram[:], [[0, 64], [1, TQ]], 0)
                        )
                        if hp0 == 0:
                            nc.vector.tensor_tensor(
                                oT[0:64, po, TQ * tq : TQ * tq + TQ],
                                opsum[tq][0:D, :],
                                rep[:],
                                ALU.mult,
                            )
                        else:
                            onrm = smp.tile([64, TQ], BF16, tag="onrm")
                            nc.vector.tensor_tensor(
                                onrm[:], opsum[tq][0:D, :], rep[:], ALU.mult
                            )
                            nc.sync.dma_start(
                                out=oT[64:128, po, TQ * tq : TQ * tq + TQ], in_=onrm[:]
                            )

            # ---- phase 3: output projection ----
            with (
                tc.tile_pool(name="wo", bufs=1) as wop,
                tc.tile_pool(name="yst", bufs=4) as ystp,
                tc.tile_pool(name="pps3", bufs=4, space="PSUM") as pps3,
            ):
                for fo2 in range(2):
                    wts = []
                    for co in range(8):
                        wt_ = wop.tile([128, TQ], BF16, tag=f"wo{co}")
                        nc.sync.dma_start(
                            out=wt_[:],
                            in_=woT[128 * co : 128 * co + 128, TQ * fo2 : TQ * fo2 + TQ],
                        )
                        wts.append(wt_)
                    for to in range(8):
                        ps = pps3.tile([128, TQ], F32, tag="pps3")
                        for co in range(8):
                            nc.tensor.matmul(
                                ps[:],
                                oT[:, co, 128 * to : 128 * to + 128],
                                wts[co][:],
                                start=(co == 0),
                                stop=(co == 7),
                            )
                        yst = ystp.tile([128, TQ], F32, tag="yst")
                        nc.vector.tensor_tensor(
                            yst[:], ps[:], borep[:, TQ * fo2 : TQ * fo2 + TQ], ALU.add
                        )
                        nc.sync.dma_start(
                            out=y_out[128 * to : 128 * to + 128, TQ * fo2 : TQ * fo2 + TQ],
                            in_=yst[:],
                        )

    _split_multi_waits(nc)
    return nc


_NC_CACHE = None


def _get_nc():
    global _NC_CACHE
    if _NC_CACHE is None:
        _NC_CACHE = _build()
    return _NC_CACHE


def _bf(x):
    return np.ascontiguousarray(np.asarray(x, np.float32).astype(ml_dtypes.bfloat16))


def _prepare_in_maps(
    query, key_, value, Wq, bq, Wk, bk, Wv, bv, Wo, bo, bias_table, offset
):
    query = np.asarray(query, np.float32)
    key_ = np.asarray(key_, np.float32)
    value = np.asarray(value, np.float32)
    shared = {
        "wqT": _bf(np.asarray(Wq, np.float32).T),
        "wkT": _bf(np.asarray(Wk, np.float32).T),
        "wvT": _bf(np.asarray(Wv, np.float32).T),
        "woT": _bf(np.asarray(Wo, np.float32).T),
        "bq2": np.ascontiguousarray(np.asarray(bq, np.float32).reshape(8, 128).T),
        "bk2": np.ascontiguousarray(np.asarray(bk, np.float32).reshape(8, 128).T),
        "bv1": np.ascontiguousarray(np.asarray(bv, np.float32)),
        "bo1": np.ascontiguousarray(np.asarray(bo, np.float32)),
        "offs": np.ascontiguousarray(np.asarray(offset, np.float32)),
    }
    tab = np.asarray(bias_table, np.float32)  # [2T-1, H]
    pad = np.concatenate([tab[0:1], tab, tab[-1:]], axis=0)  # [2T+1, H]
    shared["rtabp"] = np.ascontiguousarray(pad[::-1].T)  # [H, 2T+1]

    in_maps = []
    for b in range(B):
        m = dict(shared)
        m["xqT"] = _bf(query[b].T)
        m["xkT"] = _bf(key_[b].T)
        m["xvT"] = _bf(value[b].T)
        in_maps.append(m)
    return in_maps


def kernel(**inputs):
    in_maps = _prepare_in_maps(
        inputs["query"], inputs["key_"], inputs["value"],
        inputs["Wq"], inputs["bq"], inputs["Wk"], inputs["bk"],
        inputs["Wv"], inputs["bv"], inputs["Wo"], inputs["bo"],
        inputs["bias_table"], inputs["offset"],
    )
    nc = _get_nc()
    res = run_bass_kernel_spmd(nc, in_maps, list(range(B)))
    out = np.stack([res.results[b]["y"] for b in range(B)], axis=0)
    return out.astype(np.float32)


# revision 10
# speedup vs baseline: 1.3701x; 1.0808x over previous
"""Trainium2 Bass kernel for CustomTemporalAttention.

B=8, T=1024, E=1024, H=16, D=64. Sharding: pure batch data-parallel across the
8 NeuronCores (core b computes batch element b end-to-end; weights and the tiny
bias table are replicated). No collectives.

Per-core math (torch Linear convention x @ W.T + b):
  qT = Wq @ query[b].T  (stored transposed: [E, T], channel-major)
  kT likewise; v in [T, E] layout augmented with a ones column per head.
  Per head h: S^T[tk, tq] = sum_d kT[d,tk] qT[d,tq]
  P^T = exp(0.125 * (S^T + 8*biasT)) via DVE add + ACT exp(scale=0.125)
  [num; den] = [v_h | 1].T @ P^T  accumulated over tk chunks (PSUM [65, 512])
  O^T_h = num / den ; y = O @ Wo.T + bo.

Matmul operands are bf16 (fp32 PSUM accumulate): full-chain numpy model gives
5.1e-3 max rel err vs the fp32 reference. bf16 restores fast-weight-load and
LDWEIGHTS/ MATMUL overlap that fp32/f32r modes forfeit.

Temporal bias: bias(q,k) = lerp(table[q - k + T-1]) with a global fractional
shift u = tanh(offset)/2.  blend[r] = a*tabp[r] + b*tabp[r+1] + c*tabp[r+2]
with a=relu(-u), b=1-|u|, c=relu(u) and tabp edge-padded — exact including the
clipped endpoints. Toeplitz tiles blend[C' - i + j] are materialized per head
as BSp[i, y] = rblend[y + i] (one overlapping-window DMA from DRAM scratch)
and read back with reversed free-dim slices (both patterns HW-verified).
"""

import sys

sys.path.insert(0, "/opt/trn_rl_repo")

import ml_dtypes
import numpy as np

import concourse.bass as bass
import concourse.mybir as mybir
import concourse.tile as tile
from concourse.bass_utils import run_bass_kernel_spmd

F32 = mybir.dt.float32
BF16 = mybir.dt.bfloat16
AF = mybir.ActivationFunctionType
ALU = mybir.AluOpType

B, T, E, H = 8, 1024, 1024, 16
D = E // H  # 64
TQ = 512
W_BSP = 1920


def _split_multi_waits(nc):
    """This walrus build accepts at most one sync-wait per instruction; hoist
    extras onto same-engine NoOp carriers placed immediately before."""
    n = 0
    for f in nc.m.functions:
        for blk in f.blocks:
            out = []
            for inst in blk.instructions:
                si = inst.sync_info
                waits = list(si.on_wait) if si and si.on_wait else []
                if len(waits) > 1:
                    for w in waits[:-1]:
                        n += 1
                        nop = mybir.InstNoOp(name=f"{inst.name}-ws{n}", ins=[], outs=[])
                        nop.engine = inst.engine
                        nop.sync_info = mybir.SyncInfo(on_wait=[w], on_update=[])
                        out.append(nop)
                    inst.sync_info = mybir.SyncInfo(
                        on_wait=[waits[-1]], on_update=list(si.on_update or [])
                    )
                out.append(inst)
            blk.instructions = out
    return n


def _craft(ap, dims, offset=None):
    c = ap.copy()
    c.ap = ap.ap.__class__(dims)
    if offset is not None:
        c.offset = offset
    return c


def _build():
    nc = bass.Bass()

    xqT = nc.declare_dram_parameter("xqT", [E, T], BF16, isOutput=False)
    xkT = nc.declare_dram_parameter("xkT", [E, T], BF16, isOutput=False)
    xvT = nc.declare_dram_parameter("xvT", [E, T], BF16, isOutput=False)
    wqT = nc.declare_dram_parameter("wqT", [E, E], BF16, isOutput=False)
    wkT = nc.declare_dram_parameter("wkT", [E, E], BF16, isOutput=False)
    wvT = nc.declare_dram_parameter("wvT", [E, E], BF16, isOutput=False)
    woT = nc.declare_dram_parameter("woT", [E, E], BF16, isOutput=False)
    bq2 = nc.declare_dram_parameter("bq2", [128, 8], F32, isOutput=False)
    bk2 = nc.declare_dram_parameter("bk2", [128, 8], F32, isOutput=False)
    bv1 = nc.declare_dram_parameter("bv1", [E], F32, isOutput=False)
    bo1 = nc.declare_dram_parameter("bo1", [E], F32, isOutput=False)
    rtabp = nc.declare_dram_parameter("rtabp", [H, 2 * T + 1], F32, isOutput=False)
    offs = nc.declare_dram_parameter("offs", [1], F32, isOutput=False)
    y_out = nc.declare_dram_parameter("y", [T, E], F32, isOutput=True)

    with tile.TileContext(nc) as tc:
        with (
            tc.tile_pool(name="persist", bufs=1) as persist,
            tc.tile_pool(name="small", bufs=1) as small,
            tc.tile_pool(name="dram", bufs=1, space="DRAM") as drp,
        ):
            qT = persist.tile([128, 8, T], BF16, tag="qT")
            kT = persist.tile([128, 8, T], BF16, tag="kT")
            vp = persist.tile([128, 8, H, D + 1], BF16, tag="vp")
            oT = persist.tile([128, 8, T], BF16, tag="oT")
            bvrep = persist.tile([128, E], F32, tag="bvrep")
            borep = persist.tile([128, E], F32, tag="borep")
            bqs = small.tile([128, 8], F32, tag="bqs")
            bks = small.tile([128, 8], F32, tag="bks")

            nc.sync.dma_start(out=bqs[:], in_=bq2[:])
            nc.sync.dma_start(out=bks[:], in_=bk2[:])
            nc.sync.dma_start(out=bvrep[:], in_=_craft(bv1[:], [[0, 128], [1, E]], 0))
            nc.sync.dma_start(out=borep[:], in_=_craft(bo1[:], [[0, 128], [1, E]], 0))

            # ---- phase 0: blended relative-position table ----
            p0ctx = tc.tile_pool(name="p0", bufs=1)
            p0 = p0ctx.__enter__()
            tab = p0.tile([H, 2 * T + 1], F32, tag="tab")
            nc.sync.dma_start(out=tab[:], in_=rtabp[:])
            off_sb = p0.tile([1, 1], F32, tag="off")
            nc.sync.dma_start(out=off_sb[:], in_=offs[None, :])
            th = p0.tile([1, 1], F32, tag="th")
            nc.scalar.activation(th[:], off_sb[:], AF.Tanh)
            w8 = p0.tile([1, 1], F32, tag="w8")
            nc.vector.tensor_scalar_mul(w8[:], th[:], 4.0)  # 8*u = 4*tanh
            abc = p0.tile([1, 3], F32, tag="abc")
            nc.vector.tensor_scalar(abc[:, 0:1], w8[:], -1.0, 0.0, ALU.mult, ALU.max)
            nc.vector.tensor_scalar(abc[:, 2:3], w8[:], 1.0, 0.0, ALU.mult, ALU.max)
            tsum = p0.tile([1, 1], F32, tag="tsum")
            nc.vector.tensor_tensor(tsum[:], abc[:, 0:1], abc[:, 2:3], ALU.add)
            nc.vector.tensor_scalar(abc[:, 1:2], tsum[:], -1.0, 8.0, ALU.mult, ALU.add)
            abc_dram = drp.tile([3], F32, tag="abc_dram")
            nc.sync.dma_start(out=abc_dram[None, :], in_=abc[:])
            abc16 = p0.tile([H, 3], F32, tag="abc16")
            nc.sync.dma_start(out=abc16[:], in_=_craft(abc_dram[:], [[0, H], [1, 3]], 0))

            nblend = 2 * T - 1
            rb = p0.tile([H, nblend], F32, tag="rb")
            rb_t = p0.tile([H, nblend], F32, tag="rb_t")
            nc.vector.tensor_scalar(rb[:], tab[:, 2 : 2 + nblend], abc16[:, 0:1], None, ALU.mult)
            nc.vector.tensor_scalar(rb_t[:], tab[:, 1 : 1 + nblend], abc16[:, 1:2], None, ALU.mult)
            nc.vector.tensor_tensor(rb[:], rb[:], rb_t[:], ALU.add)
            nc.vector.tensor_scalar(rb_t[:], tab[:, 0:nblend], abc16[:, 2:3], None, ALU.mult)
            nc.vector.tensor_tensor(rb[:], rb[:], rb_t[:], ALU.add)
            rb_dram = drp.tile([H, nblend], F32, tag="rb_dram")
            nc.sync.dma_start(out=rb_dram[:], in_=rb[:])
            p0ctx.__exit__(None, None, None)

            # ---- phase 1: projections ----
            with (
                tc.tile_pool(name="xt", bufs=2) as xtp,
                tc.tile_pool(name="wt", bufs=10) as wtp,
                tc.tile_pool(name="wtv", bufs=1) as wtvp,
                tc.tile_pool(name="pps", bufs=4, space="PSUM") as pps,
            ):
                for name, x_in, w_in, dst, bias_sb in (
                    ("q", xqT, wqT, qT, bqs),
                    ("k", xkT, wkT, kT, bks),
                ):
                    xt = []
                    for eo in range(8):
                        for tq in range(2):
                            t_ = xtp.tile([128, TQ], BF16, tag=f"xt{eo}_{tq}")
                            nc.sync.dma_start(
                                out=t_[:],
                                in_=x_in[128 * eo : 128 * eo + 128, TQ * tq : TQ * tq + TQ],
                            )
                            xt.append(t_)
                    for fo in range(8):
                        ps = [pps.tile([128, TQ], F32, tag="pps", name=f"pp{fo}_{i}") for i in range(2)]
                        for eo in range(8):
                            wt_ = wtp.tile([128, 128], BF16, tag="wt")
                            nc.sync.dma_start(
                                out=wt_[:],
                                in_=w_in[128 * eo : 128 * eo + 128, 128 * fo : 128 * fo + 128],
                            )
                            for tq in range(2):
                                nc.tensor.matmul(
                                    ps[tq][:],
                                    wt_[:],
                                    xt[2 * eo + tq][:],
                                    start=(eo == 0),
                                    stop=(eo == 7),
                                )
                        for tq in range(2):
                            nc.vector.tensor_scalar(
                                dst[:, fo, TQ * tq : TQ * tq + TQ],
                                ps[tq][:],
                                1.0,
                                bias_sb[:, fo : fo + 1],
                                ALU.mult,
                                ALU.add,
                            )

                xt = []
                for eo in range(8):
                    for to2 in range(2):
                        t_ = xtp.tile([128, TQ], BF16, tag=f"xt{eo}_{to2}")
                        nc.sync.dma_start(
                            out=t_[:],
                            in_=xvT[128 * eo : 128 * eo + 128, TQ * to2 : TQ * to2 + TQ],
                        )
                        xt.append(t_)
                for fv in range(2):
                    wts = []
                    for eo in range(8):
                        wt_ = wtvp.tile([128, TQ], BF16, tag=f"wtv{eo}")
                        nc.sync.dma_start(
                            out=wt_[:],
                            in_=wvT[128 * eo : 128 * eo + 128, TQ * fv : TQ * fv + TQ],
                        )
                        wts.append(wt_)
                    for to in range(8):
                        to2, toi = divmod(to, 4)
                        ps = pps.tile([128, TQ], F32, tag="pps")
                        for eo in range(8):
                            nc.tensor.matmul(
                                ps[:],
                                xt[2 * eo + to2][:, 128 * toi : 128 * toi + 128],
                                wts[eo][:],
                                start=(eo == 0),
                                stop=(eo == 7),
                            )
                        nc.vector.tensor_tensor(
                            vp[:, to, 8 * fv : 8 * fv + 8, 0:D],
                            ps[:].rearrange("p (h d) -> p h d", d=D),
                            bvrep[:, TQ * fv : TQ * fv + TQ].rearrange(
                                "p (h d) -> p h d", d=D
                            ),
                            ALU.add,
                        )
                nc.vector.memset(vp[:, :, :, D : D + 1], 1.0)

            # ---- phase 2: attention ----
            # exp(0.125*(S + 8b)) = exp(0.125*S) * exp(b): the Toeplitz bias is
            # applied multiplicatively with a per-head exp(b) slab (bf16, DVE
            # 4x mode) instead of an fp32 PSUM add, and PV matmuls are emitted
            # as a block after the S block so the PE stream never stalls on
            # the exp chain.
            with (
                tc.tile_pool(name="bsp", bufs=2) as bspp,
                tc.tile_pool(name="eb", bufs=2) as ebp,
                tc.tile_pool(name="pt", bufs=18) as ptp,
                tc.tile_pool(name="pt0", bufs=4) as pt0p,
                tc.tile_pool(name="sm", bufs=4) as smp,
                tc.tile_pool(name="sps", bufs=5, space="PSUM") as sps,
                tc.tile_pool(name="ops", bufs=3, space="PSUM") as ops,
                tc.tile_pool(name="dr2", bufs=2, space="DRAM") as drp2,
            ):
                for h in range(H):
                    hp0 = 64 * (h % 2)
                    po = h // 2
                    bsp = bspp.tile([128, W_BSP], F32, tag="bsp")
                    nc.sync.dma_start(
                        out=bsp[:],
                        in_=_craft(rb_dram[:], [[1, 128], [1, W_BSP]], h * nblend),
                    )
                    eb = ebp.tile([128, W_BSP], BF16, tag="eb")
                    nc.scalar.activation(eb[:], bsp[:], AF.Exp, scale=0.125)
                    opsum = [
                        ops.tile([D + 1, TQ], F32, tag="ops", name=f"op{h}_{i}")
                        for i in range(2)
                    ]
                    pts = []
                    for c in range(8):
                        for tq in range(2):
                            spsum = sps.tile([128, TQ], F32, tag="sps")
                            nc.tensor.matmul(
                                spsum[:],
                                kT[hp0 : hp0 + 64, po, 128 * c : 128 * c + 128],
                                qT[hp0 : hp0 + 64, po, TQ * tq : TQ * tq + TQ],
                                start=True,
                                stop=True,
                            )
                            pt0 = pt0p.tile([128, TQ], BF16, tag="pt0")
                            nc.scalar.activation(pt0[:], spsum[:], AF.Exp, scale=0.125)
                            s = 1023 + 128 * c - TQ * tq
                            pt = ptp.tile([128, TQ], BF16, tag="pt")
                            nc.vector.tensor_tensor(
                                pt[:],
                                pt0[:],
                                eb[:, s - (TQ - 1) : s + 1][:, ::-1],
                                ALU.mult,
                            )
                            pts.append((pt, c, tq))
                    for pt, c, tq in sorted(pts, key=lambda x: (x[2], x[1])):
                        nc.tensor.matmul(
                            opsum[tq][:],
                            vp[:, c, h, :],
                            pt[:],
                            start=(c == 0),
                            stop=(c == 7),
                        )
                    for tq in range(2):
                        rec = smp.tile([1, TQ], F32, tag="rec")
                        nc.vector.reciprocal(rec[:], opsum[tq][D : D + 1, :])
                        rec_dram = drp2.tile([TQ], F32, tag="recd")
                        nc.sync.dma_start(out=rec_dram[None, :], in_=rec[:])
                        rep = smp.tile([64, TQ], F32, tag="rep")
                        nc.sync.dma_start(
                            out=rep[:], in_=_craft(rec_dram[:], [[0, 64], [1, TQ]], 0)
                        )
                        if hp0 == 0:
                            nc.vector.tensor_tensor(
                                oT[0:64, po, TQ * tq : TQ * tq + TQ],
                                opsum[tq][0:D, :],
                                rep[:],
                                ALU.mult,
                            )
                        else:
                            onrm = smp.tile([64, TQ], BF16, tag="onrm")
                            nc.vector.tensor_tensor(
                                onrm[:], opsum[tq][0:D, :], rep[:], ALU.mult
                            )
                            nc.sync.dma_start(
                                out=oT[64:128, po, TQ * tq : TQ * tq + TQ], in_=onrm[:]
                            )

            # ---- phase 3: output projection ----
            with (
                tc.tile_pool(name="wo", bufs=1) as wop,
                tc.tile_pool(name="yst", bufs=4) as ystp,
                tc.tile_pool(name="pps3", bufs=4, space="PSUM") as pps3,
            ):
                for fo2 in range(2):
                    wts = []
                    for co in range(8):
                        wt_ = wop.tile([128, TQ], BF16, tag=f"wo{co}")
                        nc.sync.dma_start(
                            out=wt_[:],
                            in_=woT[128 * co : 128 * co + 128, TQ * fo2 : TQ * fo2 + TQ],
                        )
                        wts.append(wt_)
                    for to in range(8):
                        ps = pps3.tile([128, TQ], F32, tag="pps3")
                        for co in range(8):
                            nc.tensor.matmul(
                                ps[:],
                                oT[:, co, 128 * to : 128 * to + 128],
                                wts[co][:],
                                start=(co == 0),
                                stop=(co == 7),
                            )
                        yst = ystp.tile([128, TQ], F32, tag="yst")
                        nc.vector.tensor_tensor(
                            yst[:], ps[:], borep[:, TQ * fo2 : TQ * fo2 + TQ], ALU.add
                        )
                        nc.sync.dma_start(
                            out=y_out[128 * to : 128 * to + 128, TQ * fo2 : TQ * fo2 + TQ],
                            in_=yst[:],
                        )

    _split_multi_waits(nc)
    return nc


_NC_CACHE = None


def _get_nc():
    global _NC_CACHE
    if _NC_CACHE is None:
        _NC_CACHE = _build()
    return _NC_CACHE


def _bf(x):
    return np.ascontiguousarray(np.asarray(x, np.float32).astype(ml_dtypes.bfloat16))


def _prepare_in_maps(
    query, key_, value, Wq, bq, Wk, bk, Wv, bv, Wo, bo, bias_table, offset
):
    query = np.asarray(query, np.float32)
    key_ = np.asarray(key_, np.float32)
    value = np.asarray(value, np.float32)
    shared = {
        "wqT": _bf(np.asarray(Wq, np.float32).T),
        "wkT": _bf(np.asarray(Wk, np.float32).T),
        "wvT": _bf(np.asarray(Wv, np.float32).T),
        "woT": _bf(np.asarray(Wo, np.float32).T),
        "bq2": np.ascontiguousarray(np.asarray(bq, np.float32).reshape(8, 128).T),
        "bk2": np.ascontiguousarray(np.asarray(bk, np.float32).reshape(8, 128).T),
        "bv1": np.ascontiguousarray(np.asarray(bv, np.float32)),
        "bo1": np.ascontiguousarray(np.asarray(bo, np.float32)),
        "offs": np.ascontiguousarray(np.asarray(offset, np.float32)),
    }
    tab = np.asarray(bias_table, np.float32)  # [2T-1, H]
    pad = np.concatenate([tab[0:1], tab, tab[-1:]], axis=0)  # [2T+1, H]
    shared["rtabp"] = np.ascontiguousarray(pad[::-1].T)  # [H, 2T+1]

    in_maps = []
    for b in range(B):
        m = dict(shared)
        m["xqT"] = _bf(query[b].T)
        m["xkT"] = _bf(key_[b].T)
        m["xvT"] = _bf(value[b].T)
        in_maps.append(m)
    return in_maps


def kernel(**inputs):
    in_maps = _prepare_in_maps(
        inputs["query"], inputs["key_"], inputs["value"],
        inputs["Wq"], inputs["bq"], inputs["Wk"], inputs["bk"],
        inputs["Wv"], inputs["bv"], inputs["Wo"], inputs["bo"],
        inputs["bias_table"], inputs["offset"],
    )
    nc = _get_nc()
    res = run_bass_kernel_spmd(nc, in_maps, list(range(B)))
    out = np.stack([res.results[b]["y"] for b in range(B)], axis=0)
    return out.astype(np.float32)
